# revision 14
# baseline (speedup 1.0000x reference)
"""Trainium2 Bass kernel for nn_DeltaEncoder.

Pipeline: delta encode along L -> BatchNorm2d(1) (global stats, training mode)
-> Linear(1, T) time expansion -> LIF multistep scan (decay_input, hard reset)
-> output spikes [B, T, C, L].

Sharding: data-parallel over batch B across 8 NeuronCores (4 rows each).
BN stats + normalization run as a host pre-pass mirroring the reference
op-for-op; the heavy part (T-expansion + 64-step LIF + 64MB of spike masks)
runs in the Bass kernel.

Per-core layout: the 4*8*4096 = 131072 sites live in one [128, 1024] tile:
partition p = b*32 + c*4 + l_hi, free = l_lo (l = l_hi*1024 + l_lo).

Scaled recurrence (the key trick): with P_i = 2^(i+1)*vpre_i and
V_i = 2^(i+1)*v_i, the LIF step vpre = (v + x)/2 becomes

    P_nb_i = d*A_i + V_{i-1}          A_i  = 2^i * w_i      (STT, 1 op)
    m_i    = P_nb_i < TH_i            TH_i = 2^i * (2-b_i)  (TS -> u8 out)
    V_i    = (P_nb_i + B_i) * m_i     B_i  = 2^i * b_i      (STT, 1 op)

Powers of two scale exactly in fp32, so decisions match the reference up to
one-rounding reassociation (~1e-7 relative); measured mismatches are a
handful out of 64M (rel err << 2e-2 gate).  m (the spike complement) is the
DMA'd output as uint8; the host flips s = 1 - m.

Because each step is 3 single-engine ops with no cross-engine hop, the 1024
site-columns are split into independent parallel chains:
  - cols [0, WV): pure VectorE chain (STT/TS/STT)
  - cols [WV, WV+WG): pure GpSimd chain (same ops)
  - cols [WV+WG, 1024): ScalarE writes d*A_i into PSUM, TensorE accumulates
    +I@V (fp32 identity matmul, exact), VectorE does only mask+reset.
Engine loads balance so the Vector engine drops from ~230us busy (baseline)
to ~its share of mask+reset work.
"""

import os

os.environ.setdefault("MYCRO_LOCAL_CACHE", "1")

import numpy as np

B, L, C, T = 32, 4096, 8, 64
NCORES = 8
BS = B // NCORES  # batch rows per core
P = 128           # partitions = BS * C * LH
LH = 4            # l_hi
FD = L // LH      # 1024, l_lo

_cache = {}


def _cfg():
    """Column split across engine paths + pipelining knobs."""
    return dict(
        wv=int(os.environ.get("KB3_WV", "0")),       # VectorE-chain cols
        wg=int(os.environ.get("KB3_WG", "256")),     # GpSimd-chain cols
        # PE path gets the rest: 1024 - wv - wg, split into pch sub-chunks
        pch=int(os.environ.get("KB3_PCH", "2")),
        dma_every=int(os.environ.get("KB3_DMAE", "2")),
        bufs=int(os.environ.get("KB3_BUFS", "4")),
        psum_bufs=int(os.environ.get("KB3_PBUFS", "4")),
        fp32r=os.environ.get("KB3_FP32R", "0") == "1",
        # engine for the G-chain mask compare: Pool TS->u8 measured 15.7ns/col
        # (firmware), so default the masks onto VectorE
        gmv=os.environ.get("KB3_GMV", "1") == "1",
        # paths whose mask runs as ScalarE Sign-ACT (u8-saturating): subset of
        # "vgp".  sign(TH-P) -> {-1,0,1} -> u8 saturates to (P < TH) exactly.
        signb=os.environ.get("KB3_SIGNB", ""),
        # split the V-chain into this many sub-chunks (1 = single chain)
        vch=int(os.environ.get("KB3_VCH", "1")),
        gch=int(os.environ.get("KB3_GCH", "1")),
    )


def _consts(w2, b2):
    """Per-step scaled constants, exact power-of-two scaling in f64->f32.

    w2/b2 are the halved encoder weights (w/2, b/2) as f32.
    A_i = 2^(i+1)*w2_i (= 2^i*w_i), B_i = 2^(i+1)*b2_i, TH_i = 2^(i+1)*(1-b2_i).
    """
    i = np.arange(T, dtype=np.float64)
    sc = np.exp2(i + 1.0)
    A = (sc * w2.astype(np.float64)).astype(np.float32)
    Bc = (sc * b2.astype(np.float64)).astype(np.float32)
    TH = (sc * (1.0 - b2.astype(np.float64))).astype(np.float32)
    return A, Bc, TH


def _build(w2, b2, cfg):
    """Build + compile the per-core Bass program."""
    import concourse.mybir as mybir
    import concourse.tile as tile
    from concourse import bacc

    f32 = mybir.dt.float32
    u8 = mybir.dt.uint8
    Alu = mybir.AluOpType
    Act = mybir.ActivationFunctionType

    A, Bc, TH = _consts(np.asarray(w2, np.float32), np.asarray(b2, np.float32))

    WV, WG = cfg["wv"], cfg["wg"]
    WP = FD - WV - WG
    assert WV >= 0 and WG >= 0 and WP >= 0
    DMAE = cfg["dma_every"]
    mmdt = mybir.dt.float32r if cfg["fp32r"] else f32

    # sub-chunk column ranges per path
    v_chunks = []
    if WV:
        vch = max(1, cfg["vch"])
        step = (WV + vch - 1) // vch
        for lo in range(0, WV, step):
            v_chunks.append((lo, min(lo + step, WV)))
    g_chunks = []
    if WG:
        gch = max(1, cfg["gch"])
        step = (WG + gch - 1) // gch
        for lo in range(WV, WV + WG, step):
            g_chunks.append((lo, min(lo + step, WV + WG)))
    pe_chunks = []
    if WP:
        pch = max(1, cfg["pch"])
        step = (WP + pch - 1) // pch
        assert step <= 512, "PSUM bank holds at most 512 f32 per partition"
        for lo in range(WV + WG, FD, step):
            pe_chunks.append((lo, min(lo + step, FD)))

    nc = bacc.Bacc("TRN2", target_bir_lowering=False, debug=False)
    dn_d = nc.dram_tensor("dn", [P, FD], f32, kind="ExternalInput").ap()
    if pe_chunks:
        eye_d = nc.dram_tensor("eye", [P, P], mmdt, kind="ExternalInput").ap()
    if cfg["signb"]:
        # per-step Sign-ACT bias thresholds: col t = TH_t, col T+t = 2^(t+1)
        thr_d = nc.dram_tensor("thr", [P, 2 * T], f32, kind="ExternalInput").ap()
    s_d = nc.dram_tensor("s", [BS, T, C, L], u8, kind="ExternalOutput").ap()

    with tile.TileContext(nc) as tc:
        with tc.tile_pool(name="persist", bufs=1) as pp, tc.tile_pool(
            name="work", bufs=cfg["bufs"]
        ) as wp, tc.tile_pool(name="psum", bufs=cfg["psum_bufs"], space="PSUM") as pq:
            dn = pp.tile([P, FD], f32, tag="dn")
            v = pp.tile([P, FD], f32, tag="v")
            nc.sync.dma_start(out=dn[:], in_=dn_d)
            if pe_chunks:
                eye = pp.tile([P, P], mmdt, tag="eye")
                nc.sync.dma_start(out=eye[:], in_=eye_d)
            thr = None
            if cfg["signb"]:
                thr = pp.tile([P, 2 * T], f32, tag="thr")
                nc.sync.dma_start(out=thr[:], in_=thr_d)
            sgrp = None
            for t in range(T):
                at, bt, tht = float(A[t]), float(Bc[t]), float(TH[t])
                thb = float(np.exp2(t + 1))  # bias-folded threshold, exact
                if t % DMAE == 0:
                    sgrp = wp.tile([P, DMAE * FD], u8, tag="sgrp")
                so = t % DMAE
                # --- PE path: ScalarE (d*A+B) -> PSUM, PE += I@v, V mask+reset
                pe_tiles = []
                for lo, hi in pe_chunks:
                    cs = slice(lo, hi)
                    pt = pq.tile([P, hi - lo], f32, tag=f"pp{lo}")
                    nc.scalar.activation(
                        pt[:], dn[:, cs], Act.Copy, bias=bt, scale=at
                    )
                    if t > 0:
                        nc.tensor.matmul(
                            pt[:], eye[:], v[:, cs],
                            start=False, stop=True, skip_group_check=True,
                        )
                    pe_tiles.append(pt)
                # --- G chain: ScalarE (d*A+B) -> SBUF, Pool TT/TS/TT ---
                g_tiles = []
                for lo, hi in g_chunks:
                    cs = slice(lo, hi)
                    hg = wp.tile([P, hi - lo], f32, tag=f"hg{lo}")
                    nc.scalar.activation(
                        hg[:], dn[:, cs], Act.Copy, bias=bt, scale=at
                    )
                    if t == 0:
                        pb = hg  # v == 0
                    else:
                        pb = wp.tile([P, hi - lo], f32, tag=f"pg{lo}")
                        nc.gpsimd.tensor_tensor(
                            pb[:], hg[:], v[:, cs], Alu.add
                        )
                    g_tiles.append((lo, hi, pb))
                for lo, hi, pb in g_tiles:
                    cs = slice(lo, hi)
                    ocs = slice(so * FD + lo, so * FD + hi)
                    if "g" in cfg["signb"]:
                        nc.scalar.activation(
                            sgrp[:, ocs], pb[:], Act.Sign,
                            bias=thr[:, T + t : T + t + 1], scale=-1.0,
                        )
                    else:
                        me = nc.vector if cfg["gmv"] else nc.gpsimd
                        me.tensor_scalar(
                            sgrp[:, ocs], pb[:], thb, None, Alu.is_lt
                        )
                    if t < T - 1:
                        nc.gpsimd.tensor_tensor(
                            v[:, cs], pb[:], sgrp[:, ocs], Alu.mult
                        )
                # --- V chain: STT / TS / STT, bias folded into threshold ---
                for lo, hi in v_chunks:
                    cs = slice(lo, hi)
                    ocs = slice(so * FD + lo, so * FD + hi)
                    pnb = wp.tile([P, hi - lo], f32, tag=f"pv{lo}")
                    if t == 0:
                        nc.vector.tensor_scalar(
                            pnb[:], dn[:, cs], at, None, Alu.mult
                        )
                    else:
                        nc.vector.scalar_tensor_tensor(
                            pnb[:], dn[:, cs], at, v[:, cs],
                            Alu.mult, Alu.add,
                        )
                    if "v" in cfg["signb"]:
                        nc.scalar.activation(
                            sgrp[:, ocs], pnb[:], Act.Sign,
                            bias=thr[:, t : t + 1], scale=-1.0,
                        )
                    else:
                        nc.vector.tensor_scalar(
                            sgrp[:, ocs], pnb[:], tht, None, Alu.is_lt
                        )
                    if t < T - 1:
                        nc.vector.scalar_tensor_tensor(
                            v[:, cs], pnb[:], bt, sgrp[:, ocs],
                            Alu.add, Alu.mult,
                        )
                # --- PE path mask+reset on VectorE ---
                for (lo, hi), pt in zip(pe_chunks, pe_tiles):
                    cs = slice(lo, hi)
                    ocs = slice(so * FD + lo, so * FD + hi)
                    if "p" in cfg["signb"]:
                        nc.scalar.activation(
                            sgrp[:, ocs], pt[:], Act.Sign,
                            bias=thr[:, T + t : T + t + 1], scale=-1.0,
                        )
                    else:
                        nc.vector.tensor_scalar(
                            sgrp[:, ocs], pt[:], thb, None, Alu.is_lt
                        )
                    if t < T - 1:
                        nc.vector.tensor_tensor(
                            v[:, cs], pt[:], sgrp[:, ocs], Alu.mult
                        )
                # --- output DMA every DMAE steps ---
                if t % DMAE == DMAE - 1:
                    t0 = t - DMAE + 1
                    for b in range(BS):
                        pslice = slice(b * (C * LH), (b + 1) * (C * LH))
                        if DMAE == 1:
                            out_ap = s_d[b, t].rearrange(
                                "c (lh ll) -> c lh ll", ll=FD
                            )
                            nc.sync.dma_start(out=out_ap, in_=sgrp[pslice, :])
                        else:
                            out_ap = s_d[b, t0 : t0 + DMAE].rearrange(
                                "t c (lh ll) -> c lh t ll", ll=FD
                            )
                            in_ap = sgrp[pslice, :].rearrange(
                                "p (t ll) -> p t ll", ll=FD
                            )
                            nc.sync.dma_start(out=out_ap, in_=in_ap)
    nc.compile()
    return nc


def _preprocess(inputs, bn_gamma, bn_beta):
    """Mirror the reference's delta + BatchNorm exactly (eager jnp)."""
    import jax
    import jax.numpy as jnp

    EPS = 1e-5
    inputs = jnp.asarray(inputs)
    bn_gamma = jnp.asarray(bn_gamma)
    bn_beta = jnp.asarray(bn_beta)
    delta = jnp.concatenate(
        [jnp.zeros_like(inputs[:, :1]), inputs[:, 1:] - inputs[:, :-1]], axis=1
    )  # [B, L, C]
    d = jnp.transpose(delta, (0, 2, 1))[:, None]  # [B, 1, C, L]
    mean = jnp.mean(d)
    var = jnp.var(d)
    d = (d - mean) * jax.lax.rsqrt(var + EPS) * bn_gamma[0] + bn_beta[0]
    d = jnp.transpose(d, (0, 2, 3, 1))  # [B, C, L, 1]
    return np.asarray(d)[..., 0]  # [B, C, L] f32


def _ensure_ntff_hook():
    """Install the axon NTFF profile hook that this image's antenv lacks,
    and skip the fish artifact upload. Only needed when KB_TRACE=1."""
    try:
        import sys
        import types

        try:
            from antenv.axon_hooks import get_axon_ntff_profile_hook  # noqa: F401

            have = True
        except ImportError:
            have = False
        if not have:
            from trn_agent_boot.trn_boot import _ntff_profile_via_ctypes

            hook = _ntff_profile_via_ctypes("/opt/axon/libaxon_pjrt.so")
            mod = types.ModuleType("antenv.axon_hooks")
            mod._hook = hook
            mod.get_axon_ntff_profile_hook = lambda: mod._hook
            mod.set_axon_ntff_profile_hook = lambda h: setattr(mod, "_hook", h)
            sys.modules["antenv.axon_hooks"] = mod
            import antenv

            antenv.axon_hooks = mod
        import concourse.bass_utils as bu

        bu.upload_artifacts = lambda tmpdir: tmpdir
    except Exception as e:  # pragma: no cover - tracing is best-effort
        print(f"[kernel] ntff hook setup failed: {e}")


def kernel(inputs, bn_gamma, bn_beta, enc_w, enc_b):
    from concourse.bass_utils import run_bass_kernel_spmd

    if os.environ.get("KB_TRACE"):
        _ensure_ntff_hook()

    dn = _preprocess(inputs, bn_gamma, bn_beta)

    w2 = np.asarray(enc_w, np.float32)[:, 0] * np.float32(0.5)
    b2 = np.asarray(enc_b, np.float32) * np.float32(0.5)

    cfg = _cfg()
    key = (w2.tobytes(), b2.tobytes(), tuple(sorted(cfg.items())))
    if key not in _cache:
        _cache[key] = _build(w2, b2, cfg)
    nc = _cache[key]

    dn8 = np.ascontiguousarray(dn.reshape(NCORES, BS, C, L)).reshape(NCORES, P, FD)
    in_maps = [{"dn": dn8[i]} for i in range(NCORES)]
    if FD - cfg["wv"] - cfg["wg"] > 0:
        eye = np.eye(P, dtype=np.float32)
        for im in in_maps:
            im["eye"] = eye
    if cfg["signb"]:
        A, Bc, TH = _consts(w2, b2)
        i = np.arange(T, dtype=np.float64)
        thr = np.broadcast_to(
            np.concatenate([TH, np.exp2(i + 1.0).astype(np.float32)]), (P, 2 * T)
        ).copy()
        for im in in_maps:
            im["thr"] = thr
    res = run_bass_kernel_spmd(
        nc,
        in_maps,
        core_ids=list(range(NCORES)),
        trace=bool(os.environ.get("KB_TRACE")),
    )
    kernel.last_results = res
    out = np.empty((B, T, C, L), np.float32)
    for i in range(NCORES):
        shard = res.results[i]["s"]
        np.subtract(
            np.float32(1.0), shard, out=out[i * BS : (i + 1) * BS],
            casting="unsafe",
        )
    return out


kernel.last_results = None


# revision 15
# speedup vs baseline: 1.3660x; 1.3660x over previous
"""Trainium2 Bass kernel for nn_DeltaEncoder.

Pipeline: delta encode along L -> BatchNorm2d(1) (global stats, training mode)
-> Linear(1, T) time expansion -> LIF multistep scan (decay_input, hard reset)
-> output spikes [B, T, C, L].

Sharding: data-parallel over batch B across 8 NeuronCores (4 rows each).
BN stats + normalization run as a host pre-pass mirroring the reference
op-for-op; the heavy part (T-expansion + 64-step LIF + 64MB of spike masks)
runs in the Bass kernel.

Per-core layout: the 4*8*4096 = 131072 sites live in one [128, 1024] tile:
partition p = b*32 + c*4 + l_hi, free = l_lo (l = l_hi*1024 + l_lo).

Scaled recurrence (the key trick): with P_i = 2^(i+1)*vpre_i and
V_i = 2^(i+1)*v_i, the LIF step vpre = (v + x)/2 becomes

    P_nb_i = d*A_i + V_{i-1}          A_i  = 2^i * w_i      (STT, 1 op)
    m_i    = P_nb_i < TH_i            TH_i = 2^i * (2-b_i)  (TS -> u8 out)
    V_i    = (P_nb_i + B_i) * m_i     B_i  = 2^i * b_i      (STT, 1 op)

Powers of two scale exactly in fp32, so decisions match the reference up to
one-rounding reassociation (~1e-7 relative); measured mismatches are a
handful out of 64M (rel err << 2e-2 gate).  m (the spike complement) is the
DMA'd output as uint8; the host flips s = 1 - m.

Because each step is 3 single-engine ops with no cross-engine hop, the 1024
site-columns are split into independent parallel chains:
  - cols [0, WV): pure VectorE chain (STT/TS/STT)
  - cols [WV, WV+WG): pure GpSimd chain (same ops)
  - cols [WV+WG, 1024): ScalarE writes d*A_i into PSUM, TensorE accumulates
    +I@V (fp32 identity matmul, exact), VectorE does only mask+reset.
Engine loads balance so the Vector engine drops from ~230us busy (baseline)
to ~its share of mask+reset work.
"""

import os

os.environ.setdefault("MYCRO_LOCAL_CACHE", "1")

import numpy as np

B, L, C, T = 32, 4096, 8, 64
NCORES = 8
BS = B // NCORES  # batch rows per core
P = 128           # partitions = BS * C * LH
LH = 4            # l_hi
FD = L // LH      # 1024, l_lo

_cache = {}


def _cfg():
    """Column split across engine paths + pipelining knobs."""
    return dict(
        wv=int(os.environ.get("KB3_WV", "0")),       # VectorE-chain cols
        wg=int(os.environ.get("KB3_WG", "384")),     # GpSimd-chain cols
        # PE path gets the rest: 1024 - wv - wg, split into pch sub-chunks
        pch=int(os.environ.get("KB3_PCH", "2")),
        dma_every=int(os.environ.get("KB3_DMAE", "2")),
        bufs=int(os.environ.get("KB3_BUFS", "4")),
        psum_bufs=int(os.environ.get("KB3_PBUFS", "4")),
        fp32r=os.environ.get("KB3_FP32R", "0") == "1",
        # engine for the G-chain mask compare: Pool TS->u8 measured 15.7ns/col
        # (firmware), so default the masks onto VectorE
        gmv=os.environ.get("KB3_GMV", "1") == "1",
        # paths whose mask runs as ScalarE Sign-ACT (u8-saturating): subset of
        # "vgp".  sign(TH-P) -> {-1,0,1} -> u8 saturates to (P < TH) exactly.
        signb=os.environ.get("KB3_SIGNB", ""),
        # split the V-chain into this many sub-chunks (1 = single chain)
        vch=int(os.environ.get("KB3_VCH", "1")),
        gch=int(os.environ.get("KB3_GCH", "1")),
    )


def _consts(w2, b2):
    """Per-step scaled constants, exact power-of-two scaling in f64->f32.

    w2/b2 are the halved encoder weights (w/2, b/2) as f32.
    A_i = 2^(i+1)*w2_i (= 2^i*w_i), B_i = 2^(i+1)*b2_i, TH_i = 2^(i+1)*(1-b2_i).
    """
    i = np.arange(T, dtype=np.float64)
    sc = np.exp2(i + 1.0)
    A = (sc * w2.astype(np.float64)).astype(np.float32)
    Bc = (sc * b2.astype(np.float64)).astype(np.float32)
    TH = (sc * (1.0 - b2.astype(np.float64))).astype(np.float32)
    return A, Bc, TH


def _build(w2, b2, cfg):
    """Build + compile the per-core Bass program."""
    import concourse.mybir as mybir
    import concourse.tile as tile
    from concourse import bacc

    f32 = mybir.dt.float32
    u8 = mybir.dt.uint8
    Alu = mybir.AluOpType
    Act = mybir.ActivationFunctionType

    A, Bc, TH = _consts(np.asarray(w2, np.float32), np.asarray(b2, np.float32))

    WV, WG = cfg["wv"], cfg["wg"]
    WP = FD - WV - WG
    assert WV >= 0 and WG >= 0 and WP >= 0
    DMAE = cfg["dma_every"]
    mmdt = mybir.dt.float32r if cfg["fp32r"] else f32

    # sub-chunk column ranges per path
    v_chunks = []
    if WV:
        vch = max(1, cfg["vch"])
        step = (WV + vch - 1) // vch
        for lo in range(0, WV, step):
            v_chunks.append((lo, min(lo + step, WV)))
    g_chunks = []
    if WG:
        gch = max(1, cfg["gch"])
        step = (WG + gch - 1) // gch
        for lo in range(WV, WV + WG, step):
            g_chunks.append((lo, min(lo + step, WV + WG)))
    pe_chunks = []
    if WP:
        pch = max(1, cfg["pch"])
        step = (WP + pch - 1) // pch
        assert step <= 512, "PSUM bank holds at most 512 f32 per partition"
        for lo in range(WV + WG, FD, step):
            pe_chunks.append((lo, min(lo + step, FD)))

    nc = bacc.Bacc("TRN2", target_bir_lowering=False, debug=False)
    dn_d = nc.dram_tensor("dn", [P, FD], f32, kind="ExternalInput").ap()
    if pe_chunks:
        eye_d = nc.dram_tensor("eye", [P, P], mmdt, kind="ExternalInput").ap()
    if cfg["signb"]:
        # per-step Sign-ACT bias thresholds: col t = TH_t, col T+t = 2^(t+1)
        thr_d = nc.dram_tensor("thr", [P, 2 * T], f32, kind="ExternalInput").ap()
    s_d = nc.dram_tensor("s", [BS, T, C, L], u8, kind="ExternalOutput").ap()

    with tile.TileContext(nc) as tc:
        with tc.tile_pool(name="persist", bufs=1) as pp, tc.tile_pool(
            name="work", bufs=cfg["bufs"]
        ) as wp, tc.tile_pool(name="psum", bufs=cfg["psum_bufs"], space="PSUM") as pq:
            dn = pp.tile([P, FD], f32, tag="dn")
            v = pp.tile([P, FD], f32, tag="v")
            nc.sync.dma_start(out=dn[:], in_=dn_d)
            if pe_chunks:
                eye = pp.tile([P, P], mmdt, tag="eye")
                nc.sync.dma_start(out=eye[:], in_=eye_d)
            thr = None
            if cfg["signb"]:
                thr = pp.tile([P, 2 * T], f32, tag="thr")
                nc.sync.dma_start(out=thr[:], in_=thr_d)
            sgrp = None
            for t in range(T):
                at, bt, tht = float(A[t]), float(Bc[t]), float(TH[t])
                thb = float(np.exp2(t + 1))  # bias-folded threshold, exact
                if t % DMAE == 0:
                    sgrp = wp.tile([P, DMAE * FD], u8, tag="sgrp")
                so = t % DMAE
                # --- PE path: ScalarE (d*A+B) -> PSUM, PE += I@v, V mask+reset
                pe_tiles = []
                for lo, hi in pe_chunks:
                    cs = slice(lo, hi)
                    pt = pq.tile([P, hi - lo], f32, tag=f"pp{lo}")
                    nc.scalar.activation(
                        pt[:], dn[:, cs], Act.Copy, bias=bt, scale=at
                    )
                    if t > 0:
                        nc.tensor.matmul(
                            pt[:], eye[:], v[:, cs],
                            start=False, stop=True, skip_group_check=True,
                        )
                    pe_tiles.append(pt)
                # --- G chain: ScalarE (d*A+B) -> SBUF, Pool TT/TS/TT ---
                g_tiles = []
                for lo, hi in g_chunks:
                    cs = slice(lo, hi)
                    hg = wp.tile([P, hi - lo], f32, tag=f"hg{lo}")
                    nc.scalar.activation(
                        hg[:], dn[:, cs], Act.Copy, bias=bt, scale=at
                    )
                    if t == 0:
                        pb = hg  # v == 0
                    else:
                        pb = wp.tile([P, hi - lo], f32, tag=f"pg{lo}")
                        nc.gpsimd.tensor_tensor(
                            pb[:], hg[:], v[:, cs], Alu.add
                        )
                    g_tiles.append((lo, hi, pb))
                for lo, hi, pb in g_tiles:
                    cs = slice(lo, hi)
                    ocs = slice(so * FD + lo, so * FD + hi)
                    if "g" in cfg["signb"]:
                        nc.scalar.activation(
                            sgrp[:, ocs], pb[:], Act.Sign,
                            bias=thr[:, T + t : T + t + 1], scale=-1.0,
                        )
                    else:
                        me = nc.vector if cfg["gmv"] else nc.gpsimd
                        me.tensor_scalar(
                            sgrp[:, ocs], pb[:], thb, None, Alu.is_lt
                        )
                    if t < T - 1:
                        nc.gpsimd.tensor_tensor(
                            v[:, cs], pb[:], sgrp[:, ocs], Alu.mult
                        )
                # --- V chain: STT / TS / STT, bias folded into threshold ---
                for lo, hi in v_chunks:
                    cs = slice(lo, hi)
                    ocs = slice(so * FD + lo, so * FD + hi)
                    pnb = wp.tile([P, hi - lo], f32, tag=f"pv{lo}")
                    if t == 0:
                        nc.vector.tensor_scalar(
                            pnb[:], dn[:, cs], at, None, Alu.mult
                        )
                    else:
                        nc.vector.scalar_tensor_tensor(
                            pnb[:], dn[:, cs], at, v[:, cs],
                            Alu.mult, Alu.add,
                        )
                    if "v" in cfg["signb"]:
                        nc.scalar.activation(
                            sgrp[:, ocs], pnb[:], Act.Sign,
                            bias=thr[:, t : t + 1], scale=-1.0,
                        )
                    else:
                        nc.vector.tensor_scalar(
                            sgrp[:, ocs], pnb[:], tht, None, Alu.is_lt
                        )
                    if t < T - 1:
                        nc.vector.scalar_tensor_tensor(
                            v[:, cs], pnb[:], bt, sgrp[:, ocs],
                            Alu.add, Alu.mult,
                        )
                # --- PE path mask+reset on VectorE ---
                for (lo, hi), pt in zip(pe_chunks, pe_tiles):
                    cs = slice(lo, hi)
                    ocs = slice(so * FD + lo, so * FD + hi)
                    if "p" in cfg["signb"]:
                        nc.scalar.activation(
                            sgrp[:, ocs], pt[:], Act.Sign,
                            bias=thr[:, T + t : T + t + 1], scale=-1.0,
                        )
                    else:
                        nc.vector.tensor_scalar(
                            sgrp[:, ocs], pt[:], thb, None, Alu.is_lt
                        )
                    if t < T - 1:
                        nc.vector.tensor_tensor(
                            v[:, cs], pt[:], sgrp[:, ocs], Alu.mult
                        )
                # --- output DMA every DMAE steps ---
                if t % DMAE == DMAE - 1:
                    t0 = t - DMAE + 1
                    for b in range(BS):
                        pslice = slice(b * (C * LH), (b + 1) * (C * LH))
                        if DMAE == 1:
                            out_ap = s_d[b, t].rearrange(
                                "c (lh ll) -> c lh ll", ll=FD
                            )
                            nc.sync.dma_start(out=out_ap, in_=sgrp[pslice, :])
                        else:
                            out_ap = s_d[b, t0 : t0 + DMAE].rearrange(
                                "t c (lh ll) -> c lh t ll", ll=FD
                            )
                            in_ap = sgrp[pslice, :].rearrange(
                                "p (t ll) -> p t ll", ll=FD
                            )
                            nc.sync.dma_start(out=out_ap, in_=in_ap)
    nc.compile()
    return nc


def _preprocess(inputs, bn_gamma, bn_beta):
    """Mirror the reference's delta + BatchNorm exactly (eager jnp)."""
    import jax
    import jax.numpy as jnp

    EPS = 1e-5
    inputs = jnp.asarray(inputs)
    bn_gamma = jnp.asarray(bn_gamma)
    bn_beta = jnp.asarray(bn_beta)
    delta = jnp.concatenate(
        [jnp.zeros_like(inputs[:, :1]), inputs[:, 1:] - inputs[:, :-1]], axis=1
    )  # [B, L, C]
    d = jnp.transpose(delta, (0, 2, 1))[:, None]  # [B, 1, C, L]
    mean = jnp.mean(d)
    var = jnp.var(d)
    d = (d - mean) * jax.lax.rsqrt(var + EPS) * bn_gamma[0] + bn_beta[0]
    d = jnp.transpose(d, (0, 2, 3, 1))  # [B, C, L, 1]
    return np.asarray(d)[..., 0]  # [B, C, L] f32


def _ensure_ntff_hook():
    """Install the axon NTFF profile hook that this image's antenv lacks,
    and skip the fish artifact upload. Only needed when KB_TRACE=1."""
    try:
        import sys
        import types

        try:
            from antenv.axon_hooks import get_axon_ntff_profile_hook  # noqa: F401

            have = True
        except ImportError:
            have = False
        if not have:
            from trn_agent_boot.trn_boot import _ntff_profile_via_ctypes

            hook = _ntff_profile_via_ctypes("/opt/axon/libaxon_pjrt.so")
            mod = types.ModuleType("antenv.axon_hooks")
            mod._hook = hook
            mod.get_axon_ntff_profile_hook = lambda: mod._hook
            mod.set_axon_ntff_profile_hook = lambda h: setattr(mod, "_hook", h)
            sys.modules["antenv.axon_hooks"] = mod
            import antenv

            antenv.axon_hooks = mod
        import concourse.bass_utils as bu

        bu.upload_artifacts = lambda tmpdir: tmpdir
    except Exception as e:  # pragma: no cover - tracing is best-effort
        print(f"[kernel] ntff hook setup failed: {e}")


def kernel(inputs, bn_gamma, bn_beta, enc_w, enc_b):
    from concourse.bass_utils import run_bass_kernel_spmd

    if os.environ.get("KB_TRACE"):
        _ensure_ntff_hook()

    dn = _preprocess(inputs, bn_gamma, bn_beta)

    w2 = np.asarray(enc_w, np.float32)[:, 0] * np.float32(0.5)
    b2 = np.asarray(enc_b, np.float32) * np.float32(0.5)

    cfg = _cfg()
    key = (w2.tobytes(), b2.tobytes(), tuple(sorted(cfg.items())))
    if key not in _cache:
        _cache[key] = _build(w2, b2, cfg)
    nc = _cache[key]

    dn8 = np.ascontiguousarray(dn.reshape(NCORES, BS, C, L)).reshape(NCORES, P, FD)
    in_maps = [{"dn": dn8[i]} for i in range(NCORES)]
    if FD - cfg["wv"] - cfg["wg"] > 0:
        eye = np.eye(P, dtype=np.float32)
        for im in in_maps:
            im["eye"] = eye
    if cfg["signb"]:
        A, Bc, TH = _consts(w2, b2)
        i = np.arange(T, dtype=np.float64)
        thr = np.broadcast_to(
            np.concatenate([TH, np.exp2(i + 1.0).astype(np.float32)]), (P, 2 * T)
        ).copy()
        for im in in_maps:
            im["thr"] = thr
    res = run_bass_kernel_spmd(
        nc,
        in_maps,
        core_ids=list(range(NCORES)),
        trace=bool(os.environ.get("KB_TRACE")),
    )
    kernel.last_results = res
    out = np.empty((B, T, C, L), np.float32)
    for i in range(NCORES):
        shard = res.results[i]["s"]
        np.subtract(
            np.float32(1.0), shard, out=out[i * BS : (i + 1) * BS],
            casting="unsafe",
        )
    return out


kernel.last_results = None
